# revision 5
# baseline (speedup 1.0000x reference)
"""Trainium2 Bass kernel for the 3-level soft decision-tree head.

Math (see reference): with pen = x,
  u1 = x @ W_final + b_final                            (B, 8)
  l1 = x @ Wp1 + u1*wu1 + b1 ; p1 = softmax(l1)         (B, 8, 6)
  u2 = p1 * u1
  l2 = x @ Wp2 + u2*wu2 + b2 ; p2 = softmax(l2)         (B, 8, 6, 4)
  u3 = p2 * u2
  out = concat([u1, u2.flat, u3.flat])                  (B, 248)

Key restructurings:
  * The u1*wu1 term is linear in x, so it folds into the level-1 weights
    host-side: Wp1_eff[i,:,c] = Wp1[i,:,c] + W_final[:,i]*wu1[i,c] (and
    b1_eff = b1 + b_final*wu1). All three linears then stack into one
    (1280, 248) matrix -> a single x @ W_all matmul feeds every level.
  * W_all is padded to 248->256 output columns: float32r matmuls stream at
    1 cycle/row only when the moving free dim is >= 256.
  * Sharding: pure data parallel, batch 16384 -> 8 x 2048 (one shard per
    NeuronCore). Weights (<2MB) are replicated. The x shard is staged
    host-side in [D, Bc] (transposed) layout so the device can DMA
    [d-on-partition] matmul operand tiles directly; bytes moved are
    identical to the natural layout.
  * Biases enter through one extra K=1 matmul (ones row x bias row).
  * Softmax skips the max-subtraction: logits are O(1) by construction
    (weights scaled 0.02), exp cannot overflow, and softmax is shift-
    invariant so the result is unchanged up to fp32 rounding.
"""

import sys

sys.path.insert(0, "/opt/trn_rl_repo")

from contextlib import ExitStack

import numpy as np

import concourse.bass as bass  # noqa: F401  (bass types used via tile/bacc)
import concourse.mybir as mybir
import concourse.tile as tile
from concourse import bacc, bass_utils

B, D = 16384, 1280
C1, C2, C3 = 8, 6, 4
N1 = C1                 # 8    level-1 (root) outputs
N2 = C1 * C2            # 48   level-2 logits
N3 = C1 * C2 * C3       # 192  level-3 logits
F = N1 + N2 + N3        # 248  output columns
FP = 256                # padded matmul free dim (f32r needs >=256 for 1 cyc/row)
NCORES = 8
BC = B // NCORES        # 2048 batch rows per core
NGROUPS = 4
R = BC // NGROUPS       # 512 rows per pipeline group
S = R // 128            # 4 subtiles of 128 rows
KC = D // 128           # 10 contraction chunks

f32 = mybir.dt.float32
f32r = mybir.dt.float32r

LAST_RESULT = None      # BassKernelResults of the most recent run (for test.py)
_CACHED_NC = None


def _build_body(ctx, tc, nc, xt, w, wu2r, br, onesr, y):
    const_pool = ctx.enter_context(tc.tile_pool(name="const", bufs=1))
    xt_pool = ctx.enter_context(tc.tile_pool(name="xt", bufs=2))
    psum_pool = ctx.enter_context(tc.tile_pool(name="ps", bufs=2, space="PSUM"))
    ep_pool = ctx.enter_context(tc.tile_pool(name="ep", bufs=2))
    out_pool = ctx.enter_context(tc.tile_pool(name="out", bufs=2))

    # Resident constants (matmul operands carry dtype f32r: same bits as f32,
    # but the BIR verifier requires producers of f32r-matmul inputs to declare
    # f32r output)
    w_sb = const_pool.tile([128, KC * FP], f32r)
    w_sb3 = w_sb[:].rearrange("p (c n) -> p c n", c=KC)
    nc.sync.dma_start(w_sb3, w.rearrange("(c p) n -> p c n", p=128).bitcast(f32r))
    wu2_sb = const_pool.tile([128, N3], f32)
    nc.sync.dma_start(wu2_sb[:], wu2r)
    br_sb = const_pool.tile([1, FP], f32r)
    nc.sync.dma_start(br_sb[:], br.bitcast(f32r))
    ones_sb = const_pool.tile([1, 128], f32r)
    nc.sync.dma_start(ones_sb[:], onesr.bitcast(f32r))

    xt_v = xt.rearrange("(c p) b -> p c b", p=128)

    for g in range(NGROUPS):
        # ---- load x^T slice for this group: [128, KC, R]
        xt_g = xt_pool.tile([128, KC * R], f32r)
        xt_g3 = xt_g[:].rearrange("p (c b) -> p c b", c=KC)
        nc.sync.dma_start(xt_g3, xt_v[:, :, g * R:(g + 1) * R].bitcast(f32r))

        # ---- matmul: raw[128b, s, 256] += xT_chunk.T @ W_chunk
        ps = psum_pool.tile([128, S * FP], f32)
        ps3 = ps[:].rearrange("p (s n) -> p s n", s=S)
        for s in range(S):
            reg = ps3[:, s, :]
            for k in range(KC):
                nc.tensor.matmul(
                    reg,
                    xt_g3[:, k, s * 128:(s + 1) * 128],
                    w_sb3[:, k, :],
                    start=(k == 0),
                    stop=False,
                )
            nc.tensor.matmul(
                reg,
                ones_sb[0:1, :],
                br_sb[0:1, :],
                start=False,
                stop=True,
            )

        # ---- epilogue on [128, S, 248] raw scores
        u1_ps = ps3[:, :, 0:N1]                      # [128,S,8] in PSUM
        r1_ps = ps3[:, :, N1:N1 + N2]                # [128,S,48]
        r2_ps = ps3[:, :, N1 + N2:F]                 # [128,S,192]

        out_sb = out_pool.tile([128, S * F], f32)
        out3 = out_sb[:].rearrange("p (s f) -> p s f", s=S)

        e1 = ep_pool.tile([128, S * N2], f32)
        e1_3 = e1[:].rearrange("p (s w) -> p s w", s=S)
        e1_4 = e1[:].rearrange("p (s g c) -> p s g c", s=S, g=C1)
        s1 = ep_pool.tile([128, S * N1], f32)
        s1_3 = s1[:].rearrange("p (s g) -> p s g", s=S)
        w1 = ep_pool.tile([128, S * N1], f32)
        w1_3 = w1[:].rearrange("p (s g) -> p s g", s=S)
        tmp2 = ep_pool.tile([128, S * N3], f32)
        tmp2_3 = tmp2[:].rearrange("p (s w) -> p s w", s=S)
        tmp2_4 = tmp2[:].rearrange("p (s g c) -> p s g c", s=S, g=N2)
        l2 = ep_pool.tile([128, S * N3], f32)
        l2_3 = l2[:].rearrange("p (s w) -> p s w", s=S)
        e2 = ep_pool.tile([128, S * N3], f32)
        e2_4 = e2[:].rearrange("p (s g c) -> p s g c", s=S, g=N2)
        s2 = ep_pool.tile([128, S * N2], f32)
        s2_3 = s2[:].rearrange("p (s g) -> p s g", s=S)
        w2 = ep_pool.tile([128, S * N2], f32)
        w2_3 = w2[:].rearrange("p (s g) -> p s g", s=S)

        # u1 -> output cols 0:8 (bias already folded in by the ones-row MM)
        nc.scalar.copy(out3[:, :, 0:N1], u1_ps)
        # e1 = exp(l1)  (level-1 logits come straight out of PSUM)
        nc.scalar.activation(e1_3, r1_ps, mybir.ActivationFunctionType.Exp)
        # s1[g] = sum_c e1[g,c]
        nc.vector.reduce_sum(s1_3, e1_4, axis=mybir.AxisListType.X)
        # w1 = u1 / s1  (softmax scale * upper), via fast reciprocal
        nc.vector.reciprocal_approx_fast(s1_3, s1_3)
        nc.vector.tensor_mul(w1_3, s1_3, u1_ps)
        # u2 = e1 * bcast6(w1) -> output cols 8:56
        u2_out = out3[:, :, N1:N1 + N2]
        u2_4 = u2_out.rearrange("p s (g c) -> p s g c", g=C1)
        nc.vector.tensor_mul(
            u2_4, e1_4, w1_3[:, :, :, None].broadcast_to([128, S, C1, C2])
        )
        # l2 = raw2 + bcast4(u2) * wu2
        nc.vector.tensor_mul(
            tmp2_4,
            u2_4.rearrange("p s g c -> p s (g c)")[:, :, :, None]
            .broadcast_to([128, S, N2, C3]),
            wu2_sb[:].rearrange("p (g c) -> p g c", g=N2)[:, None, :, :]
            .broadcast_to([128, S, N2, C3]),
        )
        nc.vector.tensor_add(l2_3, r2_ps, tmp2_3)
        # e2 = exp(l2)
        nc.scalar.activation(e2[:], l2[:], mybir.ActivationFunctionType.Exp)
        # s2[g] = sum_c e2[g,c] ; w2 = u2 / s2
        nc.vector.reduce_sum(s2_3, e2_4, axis=mybir.AxisListType.X)
        nc.vector.reciprocal_approx_fast(s2_3, s2_3)
        nc.vector.tensor_mul(w2_3, s2_3, u2_4.rearrange("p s g c -> p s (g c)"))
        # u3 = e2 * bcast4(w2) -> output cols 56:248
        u3_4 = out3[:, :, N1 + N2:F].rearrange("p s (g c) -> p s g c", g=N2)
        nc.vector.tensor_mul(
            u3_4, e2_4, w2_3[:, :, :, None].broadcast_to([128, S, N2, C3])
        )

        # ---- store [128, S, 248] -> y rows g*512 .. g*512+511
        y_g = y[g * R:(g + 1) * R, :].rearrange("(s p) f -> p s f", p=128)
        nc.sync.dma_start(y_g, out3)


def _get_nc():
    global _CACHED_NC
    if _CACHED_NC is not None:
        return _CACHED_NC
    nc = bacc.Bacc("TRN2", target_bir_lowering=False, debug=False,
                   num_devices=NCORES)
    xt = nc.dram_tensor("xt", [D, BC], f32, kind="ExternalInput").ap()
    w = nc.dram_tensor("w", [D, FP], f32, kind="ExternalInput").ap()
    wu2r = nc.dram_tensor("wu2r", [128, N3], f32, kind="ExternalInput").ap()
    br = nc.dram_tensor("br", [1, FP], f32, kind="ExternalInput").ap()
    onesr = nc.dram_tensor("onesr", [1, 128], f32, kind="ExternalInput").ap()
    y = nc.dram_tensor("y", [BC, F], f32, kind="ExternalOutput").ap()
    with tile.TileContext(nc) as tc, ExitStack() as ctx:
        _build_body(ctx, tc, nc, xt, w, wu2r, br, onesr, y)
    nc.compile()
    _CACHED_NC = nc
    return nc


def kernel(x, W_final, b_final, Wp1, wu1, b1, Wp2, wu2, b2):
    global LAST_RESULT
    x = np.asarray(x, np.float32)
    W_final = np.asarray(W_final, np.float64)
    b_final = np.asarray(b_final, np.float64)
    Wp1 = np.asarray(Wp1, np.float64)
    wu1 = np.asarray(wu1, np.float64)
    b1 = np.asarray(b1, np.float64)
    Wp2 = np.asarray(Wp2, np.float64)
    wu2 = np.asarray(wu2, np.float64)
    b2 = np.asarray(b2, np.float64)

    # Fold the (linear-in-x) level-1 upper term into the stacked weights.
    Wp1_eff = Wp1 + W_final.T[:, :, None] * wu1[:, None, :]     # (C1, D, C2)
    b1_eff = b1 + b_final[:, None] * wu1                        # (C1, C2)
    W_all = np.concatenate(
        [
            W_final,                                            # (D, 8)
            Wp1_eff.transpose(1, 0, 2).reshape(D, N2),          # (D, 48)
            Wp2.transpose(2, 0, 1, 3).reshape(D, N3),           # (D, 192)
        ],
        axis=1,
    )
    w_pad = np.zeros((D, FP), np.float32)
    w_pad[:, :F] = W_all.astype(np.float32)
    bias_row = np.zeros((1, FP), np.float32)
    bias_row[0, :N1] = b_final
    bias_row[0, N1:N1 + N2] = b1_eff.reshape(-1)
    bias_row[0, N1 + N2:F] = b2.reshape(-1)
    wu2_rep = np.tile(wu2.reshape(1, N3).astype(np.float32), (128, 1))

    nc = _get_nc()
    in_maps = []
    for c in range(NCORES):
        xts = np.ascontiguousarray(x[c * BC:(c + 1) * BC, :].T)
        in_maps.append({"xt": xts, "w": w_pad, "wu2r": wu2_rep, "br": bias_row,
                        "onesr": np.ones((1, 128), np.float32)})
    res = bass_utils.run_bass_kernel_spmd(nc, in_maps, core_ids=list(range(NCORES)))
    LAST_RESULT = res
    return np.concatenate([res.results[c]["y"] for c in range(NCORES)], axis=0)
